# revision 1
# baseline (speedup 1.0000x reference)
"""Bass/Trainium2 kernel for nn_EquivariantProductBasisBlock.

Math (per node n, feature f):
    s = x[n,f,0]; v = x[n,f,1:4]; vv = (v.v)/sqrt(3)
    out0 = a0[sp,0]*s + a0[sp,1]*s^2 + a0[sp,2]*vv + a0[sp,3]*s^3 + a0[sp,4]*(s*vv)
    c1   = a1[sp,0] + a1[sp,1]*s + a1[sp,2]*s^2 + a1[sp,3]*vv
    y0 = out0 @ W0 / 16 ;  y1_c = (c1*v_c) @ W1 / 16
    out = concat(y0, y1) over the lm axis.

Strategy: shard nodes over 8 cores. Host sorts nodes by species so the
species-dependent path weights become per-partition scalar columns on
device (features on partitions, nodes on the free axis). The 1/sqrt(3)
and 1/16 factors are folded into the weight tables on the host. vv is
precomputed on the host and shipped as a component alongside s (order
s, vv, vx, vy, vz so [s;vv] and [out0a;tt] pair into single DVE ops).

Engine assignment (gpsimd is left idle: it shares SBUF ports with DVE,
so offloading elementwise work there just stalls DVE):
    DVE : bb/gg/w_/h4 tensor_scalar affines (4x bf16) + paired
          broadcasts [h3,p1]=s*[h2,gg], [out0a,tt]=[s,vv]*[h4,bb],
          c1=p1+w_, rhs=c1*v (2x bf16)
    ACT : h2 affine (queued ahead of copies) + PSUM->SBUF copies +
          output DMA issue
    PE  : 4 GEMMs (comps 1,2,3 then 0; comp0 movings = [out0a, tt])
    SP  : input DMA issue only (deep prefetch, never blocked)
    I/O : flat per-block-contiguous DRAM layouts

Polynomial factorization (per-species coefficients fold into
per-partition scalars):
    h2 = a3*s + a1 ; gg = a12*s + a11 ; bb = a4*s + a2 ; w = a13*vv + a10
    [h3, p1] = s * [h2, gg] ; h4 = h3 + a0
    [out0a, tt] = [s, vv] * [h4, bb]   (out0 = out0a + tt in PSUM)
    c1 = p1 + w ; rhs_c = c1 * v_c
"""

import numpy as np
from contextlib import ExitStack

import ml_dtypes

N_CORES = 8
F = 256
NUM_SPECIES = 10
NB = 1024   # nodes per compute block
SUB = 512   # matmul moving free-dim limit
INV_SQRT3 = 1.0 / np.sqrt(3.0)
INV_SQRT_F = 1.0 / np.sqrt(256.0)

_KERNEL_CACHE = {}


def _make_blocks(ntot):
    """ntot is a multiple of SUB. Small first/last blocks for pipeline
    ramp-in/ramp-out."""
    head = [256, 256, 512]
    tail = [512, 256, 256]
    blocks = []
    j = 0
    for nb in head:
        blocks.append((j, nb))
        j += nb
    mid_end = ntot - sum(tail)
    while mid_end - j >= NB:
        blocks.append((j, NB))
        j += NB
    while j < mid_end:
        nb = min(SUB, mid_end - j)
        blocks.append((j, nb))
        j += nb
    for nb in tail:
        blocks.append((j, nb))
        j += nb
    return blocks


def _build_bass(c_sp, ntot):
    """Build + compile the per-core Bass graph.

    c_sp: per-species padded segment length (same on every core), even.
    ntot: total padded nodes per core (multiple of SUB).
    """
    import concourse.bacc as bacc
    import concourse.bass as bass
    import concourse.mybir as mybir
    import concourse.tile as tile

    fp32 = mybir.dt.float32
    bf16 = mybir.dt.bfloat16
    AF = mybir.ActivationFunctionType
    OP = mybir.AluOpType

    nc = bacc.Bacc("TRN2", target_bir_lowering=False, debug=False)

    # flat per-block-contiguous layouts: x block slab = [128, 5*2*nb],
    # y block slab = [128, 4*2*nb]
    x = nc.dram_tensor("x", [128, 10 * ntot], bf16, kind="ExternalInput")
    a0 = nc.dram_tensor("a0", [256, 5 * NUM_SPECIES], fp32, kind="ExternalInput")
    a1 = nc.dram_tensor("a1", [256, 4 * NUM_SPECIES], fp32, kind="ExternalInput")
    w0 = nc.dram_tensor("w0", [256, 256], bf16, kind="ExternalInput")
    w1 = nc.dram_tensor("w1", [256, 256], bf16, kind="ExternalInput")
    y = nc.dram_tensor("y", [128, 8 * ntot], bf16, kind="ExternalOutput")

    blocks = _make_blocks(ntot)
    ends = np.cumsum(c_sp)

    def segments(j0, nb):
        segs = []
        for sp in range(NUM_SPECIES):
            lo = int(ends[sp] - c_sp[sp])
            hi = int(ends[sp])
            a = max(lo, j0)
            b = min(hi, j0 + nb)
            if a < b:
                segs.append((sp, a - j0, b - a))
        return segs

    with tile.TileContext(nc) as tc:
        with ExitStack() as ctx:
            consts = ctx.enter_context(tc.tile_pool(name="consts", bufs=1))
            io_in = ctx.enter_context(tc.tile_pool(name="io_in", bufs=4))
            mv_p = ctx.enter_context(tc.tile_pool(name="mv", bufs=2))
            tmp1 = ctx.enter_context(tc.tile_pool(name="tmp1", bufs=1))
            tmp2 = ctx.enter_context(tc.tile_pool(name="tmp2", bufs=2))
            stag = ctx.enter_context(tc.tile_pool(name="stag", bufs=2))
            psum = ctx.enter_context(tc.tile_pool(name="psum", bufs=2, space="PSUM"))

            # --- constants (DMAs issued after the first input block below) ---
            w0_sb = consts.tile([128, 2, 256], bf16)
            w1_sb = consts.tile([128, 2, 256], bf16)
            a0_sb = consts.tile([128, 2, 5 * NUM_SPECIES], fp32)
            a1_sb = consts.tile([128, 2, 4 * NUM_SPECIES], fp32)
            # const DMAs staged on the SP queue: scalar tables right after
            # the first input block (DVE/ACT need them first), weight
            # matrices after the second (PE needs them much later).
            # Issuing them from the ACT queue instead measured worse (ACT
            # is a co-pacemaker; its queue-head time is more precious than
            # the SP serialization this avoids).
            consts_stage = [0]

            def emit_const_dmas():
                if consts_stage[0] == 0:
                    nc.sync.dma_start(out=a0_sb, in_=a0[:].rearrange("(fc p) c -> p fc c", p=128))
                    nc.sync.dma_start(out=a1_sb, in_=a1[:].rearrange("(fc p) c -> p fc c", p=128))
                elif consts_stage[0] == 1:
                    nc.sync.dma_start(out=w0_sb, in_=w0[:].rearrange("(fc p) g -> p fc g", p=128))
                    nc.sync.dma_start(out=w1_sb, in_=w1[:].rearrange("(fc p) g -> p fc g", p=128))
                consts_stage[0] += 1

            def a0c(fc, sp, p):
                i = sp * 5 + p
                return a0_sb[:, fc, i : i + 1]

            def a1c(fc, sp, p):
                i = sp * 4 + p
                return a1_sb[:, fc, i : i + 1]

            def emit_input_h2(j0, nb):
                """SP input DMA + ACT h2 affine (one pipeline stage early)."""
                segs = segments(j0, nb)

                xin = io_in.tile([128, 5, 2, nb], bf16, tag="xin", name=f"xin_{j0}")
                nc.sync.dma_start(
                    out=xin.rearrange("p c f n -> p (c f n)"),
                    in_=x[:, 10 * j0 : 10 * (j0 + nb)],
                )
                emit_const_dmas()

                # host component order: s, vv, vx, vy, vz
                s_all = xin[:, 0, :, :]     # [128, 2, nb]
                vv = xin[:, 1, :, :]        # [128, 2, nb]

                hg = tmp2.tile([128, 2, 2, nb], bf16, tag="hg")
                h2 = hg[:, 0]
                w_ = tmp2.tile([128, 2, nb], bf16, tag="w_")

                # h2 and w affines on ACT (a full block ahead of their DVE
                # consumers; ACT has slack next to its PSUM copies)
                for fc in range(2):
                    for (sp, o, L) in segs:
                        sl = slice(o, o + L)
                        nc.scalar.activation(
                            h2[:, fc, sl], s_all[:, fc, sl], AF.Identity,
                            bias=a0c(fc, sp, 1), scale=a0c(fc, sp, 3),
                        )
                        nc.scalar.activation(
                            w_[:, fc, sl], vv[:, fc, sl], AF.Identity,
                            bias=a1c(fc, sp, 0), scale=a1c(fc, sp, 3),
                        )

                return dict(j0=j0, nb=nb, xin=xin, hg=hg, w_=w_, segs=segs)

            def emit_dve(st):
                """DVE pointwise work for a block whose input + h2 are ready."""
                j0, nb, xin, hg, segs = st["j0"], st["nb"], st["xin"], st["hg"], st["segs"]

                s_all = xin[:, 0, :, :]     # [128, 2, nb]
                vv = xin[:, 1, :, :]        # [128, 2, nb] host-precomputed v.v
                svv = xin[:, 0:2, :, :]     # [128, 2, 2, nb] pair [s, vv]
                v_all = xin[:, 2:5, :, :]   # [128, 3, 2, nb]

                gg = hg[:, 1]
                w_ = st["w_"]
                hp = tmp1.tile([128, 2, 2, nb], bf16, tag="hp")
                h3 = hp[:, 0]
                p1 = hp[:, 1]
                h4bb = tmp1.tile([128, 2, 2, nb], bf16, tag="h4bb")
                h4 = h4bb[:, 0]
                bb = h4bb[:, 1]
                c1 = tmp1.tile([128, 2, nb], bf16, tag="c1")
                o0t = tmp1.tile([128, 2, 2, nb], bf16, tag="o0t")
                o0s = mv_p.tile([128, 2, nb], bf16, tag="o0s", name=f"o0s_{j0}")
                rhs = mv_p.tile([128, 3, 2, nb], bf16, tag="rhs", name=f"rhs_{j0}")

                # per-species affines on DVE tensor_scalar (4x bf16 mode)
                for fc in range(2):
                    for (sp, o, L) in segs:
                        sl = slice(o, o + L)
                        nc.vector.tensor_scalar(
                            gg[:, fc, sl], s_all[:, fc, sl],
                            a1c(fc, sp, 2), a1c(fc, sp, 1), OP.mult, OP.add,
                        )
                        nc.vector.tensor_scalar(
                            bb[:, fc, sl], s_all[:, fc, sl],
                            a0c(fc, sp, 4), a0c(fc, sp, 2), OP.mult, OP.add,
                        )

                # [h3, p1] = s * [h2, gg]  (one op, s broadcast over the pair)
                s_b2 = bass.AP(
                    tensor=xin.tensor,
                    offset=s_all.offset,
                    ap=[s_all.ap[0], [0, 2], s_all.ap[1], s_all.ap[2]],
                )
                nc.vector.tensor_tensor(hp, s_b2, hg, OP.mult)

                for fc in range(2):
                    for (sp, o, L) in segs:
                        sl = slice(o, o + L)
                        nc.vector.tensor_scalar(
                            h4[:, fc, sl], h3[:, fc, sl], a0c(fc, sp, 0), None, OP.add,
                        )

                # [out0a, tt] = [s, vv] * [h4, bb]  (one paired op), then
                # out0 = out0a + tt pre-added so comp0 needs one PE moving
                # (saves 20% of PE flops + 20% of ldweights -> less power)
                nc.vector.tensor_tensor(o0t, svv, h4bb, OP.mult)
                nc.vector.tensor_tensor(o0s, o0t[:, 0], o0t[:, 1], OP.add)

                nc.vector.tensor_tensor(c1, hp[:, 1], w_, OP.add)

                # out1 = c1 * v  (c1 broadcast over the 3 components)
                c1b = bass.AP(
                    tensor=c1.tensor,
                    offset=c1.offset,
                    ap=[c1.ap[0], [0, 3], c1.ap[1], c1.ap[2]],
                )
                nc.vector.tensor_tensor(rhs, c1b, v_all, OP.mult)

                st["o0s"] = o0s
                st["rhs"] = rhs

            def emit_gemm(st):
                """PE GEMMs + ACT copies + output DMA for a previous block."""
                j0, nb = st["j0"], st["nb"]
                stg = stag.tile([128, 4, 2, nb], bf16, tag="stg", name=f"stg_{j0}")
                nsub = (nb + SUB - 1) // SUB
                rhs = st["rhs"]
                movings = {
                    0: [st["o0s"]],
                    1: [rhs[:, 0]],
                    2: [rhs[:, 1]],
                    3: [rhs[:, 2]],
                }
                for comp in (1, 2, 3, 0):
                    ps = psum.tile([128, 2, nb], fp32, tag="ps", name=f"ps{comp}_{j0}")
                    w_sb = w0_sb if comp == 0 else w1_sb
                    mvs = movings[comp]
                    for gc in range(2):
                        g0 = gc * 128
                        for fc in range(2):
                            lhsT = w_sb[:, fc, g0 : g0 + 128]
                            for mi, mv in enumerate(mvs):
                                for si in range(nsub):
                                    o = si * SUB
                                    L = min(SUB, nb - o)
                                    nc.tensor.matmul(
                                        ps[:, gc, o : o + L],
                                        lhsT,
                                        mv[:, fc, o : o + L],
                                        start=(fc == 0 and mi == 0),
                                        stop=(fc == 1 and mi == len(mvs) - 1),
                                    )
                    nc.scalar.activation(stg[:, comp], ps, AF.Copy)

                # output DMA from the ACT queue, right after the copies: the
                # natural pacing (one transfer per copy batch) keeps DMA
                # traffic smooth; issuing these from the idle gpsimd queue
                # measures faster on typical cores but bursts transfers and
                # widens the max-core tail, which is the graded metric
                nc.scalar.dma_start(
                    out=y[:, 8 * j0 : 8 * (j0 + nb)],
                    in_=stg.rearrange("p c g n -> p (c g n)"),
                )

            # 3-stage pipeline: input+h2(i) | DVE(i-1) | GEMM+store(i-2)
            sts = []
            for i, (j0, nb) in enumerate(blocks):
                if i >= 2:
                    emit_gemm(sts[i - 2])
                sts.append(emit_input_h2(j0, nb))
                if i >= 1:
                    emit_dve(sts[i - 1])
            emit_dve(sts[-1])
            if len(sts) >= 2:
                emit_gemm(sts[-2])
            emit_gemm(sts[-1])

    nc.compile()
    return nc


def _prepare(node_feats, node_specie, w0, w1, W0, W1):
    """Host-side: sort by species, shard, transpose, fold scale factors."""
    sp = np.asarray(node_specie).astype(np.int64)

    ids_by_sp = [np.nonzero(sp == s)[0] for s in range(NUM_SPECIES)]
    core_ids = [[ids_by_sp[s][c::N_CORES] for s in range(NUM_SPECIES)] for c in range(N_CORES)]
    # even segment lengths keep bf16 slices 4B-aligned on device
    c_sp = [
        (max(len(core_ids[c][s]) for c in range(N_CORES)) + 1) // 2 * 2
        for s in range(NUM_SPECIES)
    ]
    ntot = int(np.sum(c_sp))
    pad_tail = (-ntot) % SUB
    c_sp[-1] += pad_tail
    ntot += pad_tail

    idx = np.zeros((N_CORES, ntot), dtype=np.int64)
    valid = np.zeros((N_CORES, ntot), dtype=bool)
    off = 0
    for s in range(NUM_SPECIES):
        L = c_sp[s]
        for c in range(N_CORES):
            ids = core_ids[c][s]
            k = len(ids)
            idx[c, off : off + k] = ids
            valid[c, off : off + k] = True
        off += L

    w0a = np.asarray(w0, np.float32).copy()
    w1a = np.asarray(w1, np.float32).copy()
    w0a[:, 2, :] *= INV_SQRT3
    w0a[:, 4, :] *= INV_SQRT3
    w1a[:, 3, :] *= INV_SQRT3
    a0_tab = np.ascontiguousarray(
        w0a.transpose(2, 0, 1).reshape(F, 5 * NUM_SPECIES)
    )
    a1_tab = np.ascontiguousarray(
        w1a.transpose(2, 0, 1).reshape(F, 4 * NUM_SPECIES)
    )
    W0s = (np.asarray(W0, np.float32) * INV_SQRT_F).astype(ml_dtypes.bfloat16)
    W1s = (np.asarray(W1, np.float32) * INV_SQRT_F).astype(ml_dtypes.bfloat16)

    nf = np.asarray(node_feats, np.float32)
    n_ = nf.shape[0]
    # component order: s, vv, vx, vy, vz
    xf = np.empty((5, F, n_), np.float32)
    xf[0] = nf[:, :, 0].T
    xf[1] = (nf[:, :, 1] ** 2 + nf[:, :, 2] ** 2 + nf[:, :, 3] ** 2).T
    xf[2:5] = nf[:, :, 1:4].transpose(2, 1, 0)
    xt = xf.astype(ml_dtypes.bfloat16)  # [5,256,n]

    blocks = _make_blocks(ntot)
    xs = []
    for c in range(N_CORES):
        xc = xt[:, :, idx[c]].reshape(5, 2, 128, ntot)  # [c, fc, p, n]
        xflat = np.empty((128, 10 * ntot), ml_dtypes.bfloat16)
        for (j0, nb) in blocks:
            blk = xc[:, :, :, j0 : j0 + nb]             # [5, 2, 128, nb]
            xflat[:, 10 * j0 : 10 * (j0 + nb)] = (
                blk.transpose(2, 0, 1, 3).reshape(128, 10 * nb)
            )
        xs.append(xflat)

    return xs, idx, valid, tuple(c_sp), ntot, blocks, a0_tab, a1_tab, W0s, W1s


def kernel(node_feats, node_specie, w0, w1, W0, W1):
    from concourse.bass_utils import run_bass_kernel_spmd

    xs, idx, valid, c_sp, ntot, blocks, a0_tab, a1_tab, W0s, W1s = _prepare(
        node_feats, node_specie, w0, w1, W0, W1
    )

    key = (c_sp, ntot)
    if key not in _KERNEL_CACHE:
        _KERNEL_CACHE[key] = _build_bass(list(c_sp), ntot)
    nc = _KERNEL_CACHE[key]

    in_maps = [
        {"x": xs[c], "a0": a0_tab, "a1": a1_tab, "w0": W0s, "w1": W1s}
        for c in range(N_CORES)
    ]
    res = run_bass_kernel_spmd(nc, in_maps, core_ids=list(range(N_CORES)))

    n = node_feats.shape[0]
    out = np.empty((n, F, 4), dtype=np.float32)
    for c in range(N_CORES):
        yflat = res.results[c]["y"]  # [128, 8*ntot] bf16
        yt = np.empty((ntot, F, 4), np.float32)
        for (j0, nb) in blocks:
            blk = yflat[:, 8 * j0 : 8 * (j0 + nb)].reshape(128, 4, 2, nb)
            # [p, comp, gc, n] -> [n, gc*128+p, comp]
            yt[j0 : j0 + nb] = (
                blk.astype(np.float32).transpose(3, 2, 0, 1).reshape(nb, F, 4)
            )
        m = valid[c]
        out[idx[c][m]] = yt[m]
    return out

